# revision 5
# baseline (speedup 1.0000x reference)
"""Trainium2 Bass kernel for the SNN (snntorch Leaky, subtract-reset) forward.

Computation (reference):
    cur1 = x @ W1.T + b1                      # [B, 100], static across steps
    25 steps of:  reset1 = H(mem1 - 1); mem1 = 0.95*mem1 + cur1 - reset1
                  spk1 = H(mem1 - 1);   cur2 = spk1 @ W2.T + b2
                  reset2 = H(mem2 - 1); mem2 = 0.95*mem2 + cur2 - reset2
    returns mem2 per step: [25, B, 2]

Device algorithm (per core, batch shard of 8192, hidden dim on partitions):
  Change of variables: A = cur1/(1-b), z = mem1 - A, rescale Zt = b^-t z_t:
    Z_t = Z_{t-1} - c_t spk_{t-1},  c_t = b^-t
    spk_t = H(b^t Z_t - th0),       th0 = 1 + Z_0  (static per element!)
  The compare is ONE DVE scalar_tensor_tensor per step (mult + is_gt against
  the static th0) -- no per-step threshold rebuild.
  The spike subtract AND cur2 = W2 spk + b2 merge into a single accumulating
  matmul per 512-col block: stationary S_t [101, 128] has diag(-c_{t+1}) in
  cols 0..99 (spike rows; ones-row coeff 0) and W2/b2 in cols 100+2g,101+2g,
  writing PSUM rows 100..127 which accumulate from virgin state (the fp32
  z0-matmul has M=128 with start=True, zeroing them). bf16 is 1 cycle/row on
  the PE (fp32 is 4): stationaries are split hi/lo (bf16 + bf16 residual, 16
  mantissa bits total) and applied as two accumulating matmuls -- spikes are
  exactly representable, so the pair reproduces the fp32-level recurrence
  (end-to-end rel err ~4.6e-3 vs fp32 ~1.3e-3, gate 2e-2).
  cur2 groups: steps 1-14 -> rows 100-127 (ACT-drained to SBUF after slot
  14), steps 15-25 re-accumulate rows 100-121 and are drained RAW; the stale
  group-1 addend is subtracted once in the tail after the PE transpose
  (transpose is linear). Engine partition access must start 32-aligned, so
  drains read zt[96:128] (4 garbage rows ride along) and the cur2 SBUF tile
  keeps them: step t lives at row 4+2(t-1) (t<=14) / 36+2(t-15) (t>=15).
  Tail: PE-transpose to batch-major [128, nj, 64], group-2 fix-up, the cheap
  mem2 recurrence (is_gt / stt / add on DVE), reorder, one output DMA.
"""

import numpy as np

BETA = 0.95
T = 25
NI, NH, NO = 9, 100, 2
B = 65536
NCORES = 8
SH = B // NCORES          # batch shard per core
G1 = 14                   # steps in cur2 group 1
f32 = np.float32

_CACHE = {}
_LAST_RESULT = None       # test.py pokes at these for its timing harness
_LAST_IN_MAPS = None


def _bf16_np(x):
    import ml_dtypes
    return np.asarray(x, np.float32).astype(ml_dtypes.bfloat16)


def _bf16_round(x):
    x = np.asarray(x, np.float32)
    b = x.view(np.uint32)
    b = ((b + 0x8000 + ((b >> 16) & 1)) & 0xFFFF0000).astype(np.uint32)
    return b.view(np.float32)


def _row_of(t):
    """cur2 SBUF row (partition) holding output neuron 0 of step t."""
    return 4 + 2 * (t - 1) if t <= G1 else 36 + 2 * (t - 1 - G1)


def _build_nop_nc():
    """Minimal kernel (one tiny DMA round-trip) for dispatch-overhead baseline."""
    import concourse.bass as bass
    import concourse.tile as tile
    from concourse import bacc, mybir
    f32d = mybir.dt.float32
    nc = bacc.Bacc("TRN2", target_bir_lowering=False, debug=False,
                   num_devices=NCORES)
    i_d = nc.dram_tensor("nin", [1, 128], f32d, kind="ExternalInput").ap()
    o_d = nc.dram_tensor("nout", [1, 128], f32d, kind="ExternalOutput").ap()
    with tile.TileContext(nc) as tc:
        with tc.tile_pool(name="sb", bufs=1) as sb:
            tl = sb.tile([1, 128], f32d)
            nc.sync.dma_start(tl[:], i_d[:])
            nc.sync.dma_start(o_d[:], tl[:])
    nc.compile()
    return nc


def _build_nc(sh, rc, n_iter=1):
    """Build + compile the Bass program for shard size `sh`, round width `rc`.

    n_iter > 1 repeats the whole computation (same buffers, serialized by
    data deps) so the marginal per-iteration cost can be measured through
    the noisy axon dispatch path. Functionally identical output.
    """
    import concourse.bass as bass
    import concourse.tile as tile
    from concourse import bacc, mybir

    f32d = mybir.dt.float32
    bf16d = mybir.dt.bfloat16
    Copy = mybir.ActivationFunctionType.Copy
    Alu = mybir.AluOpType
    nr = sh // rc             # rounds
    nj = sh // 128            # 128-column groups (transpose tiles)
    nblk = rc // 512          # 512-col matmul blocks per round

    nc = bacc.Bacc("TRN2", target_bir_lowering=False, debug=False,
                   num_devices=NCORES)

    xt_d = nc.dram_tensor("xt", [NI + 1, sh], f32d, kind="ExternalInput").ap()
    iw_d = nc.dram_tensor("iw", [NI + 1, 128], f32d,
                          kind="ExternalInput").ap()
    shi_d = nc.dram_tensor("shi", [NH + 1, T * 128], bf16d,
                           kind="ExternalInput").ap()
    slo_d = nc.dram_tensor("slo", [NH + 1, T * 128], bf16d,
                           kind="ExternalInput").ap()
    on_d = nc.dram_tensor("ones", [1, rc], bf16d, kind="ExternalInput").ap()
    id_d = nc.dram_tensor("ident", [128, 128], f32d, kind="ExternalInput").ap()
    out_d = nc.dram_tensor("out", [T, sh, NO], f32d, kind="ExternalOutput").ap()

    half = rc // 2
    with tile.TileContext(nc) as tc:
        with tc.tile_pool(name="const", bufs=1) as cp, \
             tc.tile_pool(name="work", bufs=2) as wp, \
             tc.tile_pool(name="big", bufs=1) as bp:
          for _it in range(n_iter):
            xt = cp.tile([NI + 1, sh], f32d, tag="xt", name="xt")
            iw = cp.tile([NI + 1, 128], f32d, tag="iw", name="iw")
            shi = cp.tile([NH + 1, T * 128], bf16d, tag="shi", name="shi")
            slo = cp.tile([NH + 1, T * 128], bf16d, tag="slo", name="slo")
            ident = cp.tile([128, 128], f32d, tag="ident", name="ident")
            cur2 = bp.tile([64, sh], f32d, tag="cur2", name="cur2")
            osb = bp.tile([128, T, nj, NO], f32d, tag="osb", name="osb")
            spks = [cp.tile([NH + 1, rc], bf16d, tag=f"spk{i}",
                            name=f"spk{i}") for i in range(2)]
            nc.sync.dma_start(xt[:], xt_d[:])
            nc.sync.dma_start(iw[:], iw_d[:])
            nc.sync.dma_start(shi[:], shi_d[:])
            nc.sync.dma_start(slo[:], slo_d[:])
            nc.sync.dma_start(ident[:], id_d[:])
            for s_ in spks:
                nc.sync.dma_start(s_[NH:NH + 1, :], on_d[:])

            ps_rounds = tc.tile_pool(name="psA", bufs=1,
                                     space=bass.MemorySpace.PSUM)
            ps = ps_rounds.__enter__()
            for r in range(nr):
                cs = slice(r * rc, (r + 1) * rc)
                zt = ps.tile([128, rc], f32d, tag="zt")
                th0 = wp.tile([NH, rc], f32d, tag="th0", name=f"th0_{r}")

                # z0 (rows 0-99) + explicit zeros into rows 100-127 so the
                # per-step accumulating matmuls see virgin cur2 rows.
                for k in range(nblk):
                    bs = slice(k * 512, (k + 1) * 512)
                    nc.tensor.matmul(zt[:, bs], iw[:],
                                     xt[:, r * rc + k * 512:
                                        r * rc + (k + 1) * 512],
                                     start=True, stop=True)
                # static threshold th0 = 1 + z0 (split to pipeline with
                # the first compares)
                nc.scalar.activation(th0[:, 0:half], zt[0:NH, 0:half], Copy,
                                     bias=1.0, scale=1.0)
                nc.scalar.activation(th0[:, half:rc], zt[0:NH, half:rc], Copy,
                                     bias=1.0, scale=1.0)

                for t in range(1, T + 1):
                    bt = float(f32(np.float64(BETA) ** t))
                    spk = spks[t % 2]
                    # spk_t = (b^t * Z_t) > th0   (two DVE instrs)
                    nc.vector.scalar_tensor_tensor(
                        spk[0:NH, 0:half], zt[0:NH, 0:half], bt,
                        th0[:, 0:half], Alu.mult, Alu.is_gt)
                    nc.vector.scalar_tensor_tensor(
                        spk[0:NH, half:rc], zt[0:NH, half:rc], bt,
                        th0[:, half:rc], Alu.mult, Alu.is_gt)
                    # merged matmul: Z -= c_{t+1} spk_t; cur2_t into group row
                    ws = slice((t - 1) * 128, t * 128)
                    for k in range(nblk):
                        bs = slice(k * 512, (k + 1) * 512)
                        nc.tensor.matmul(zt[:, bs], shi[:, ws], spk[:, bs],
                                         start=False, stop=False,
                                         skip_group_check=True)
                        nc.tensor.matmul(zt[:, bs], slo[:, ws], spk[:, bs],
                                         start=False, stop=True,
                                         skip_group_check=True)
                        if t == G1 or t == T:
                            ro = 0 if t == G1 else 32
                            nc.scalar.activation(
                                cur2[ro:ro + 32,
                                     r * rc + k * 512:r * rc + (k + 1) * 512],
                                zt[96:128, bs], Copy, bias=0.0, scale=1.0)

            ps_rounds.__exit__(None, None, None)

            # ---- tail: transpose, group-2 fix, mem2 recurrence, output ----
            ps_m2 = tc.tile_pool(name="psB", bufs=1,
                                 space=bass.MemorySpace.PSUM)
            ps2 = ps_m2.__enter__()
            m2 = ps2.tile([128, nj, 64], f32d, tag="m2")
            for j in range(nj):
                nc.tensor.transpose(m2[:, j, 0:64],
                                    cur2[:, j * 128:(j + 1) * 128],
                                    ident[0:64, 0:64])
            # group-2 raw drains contain the stale group-1 values: subtract
            # (transpose is linear, so fixing up batch-major is equivalent).
            # DVE can read only one PSUM operand -> snapshot the stale cols
            # to SBUF first (also needed because the recurrence below mutates
            # those rows in place).
            stale = bp.tile([128, nj, 22], f32d, tag="stale", name="stale")
            nc.scalar.activation(stale[:], m2[:, :, 4:26], Copy, bias=0.0,
                                 scale=1.0)
            nc.vector.tensor_tensor(m2[:, :, 36:58], m2[:, :, 36:58],
                                    stale[:], Alu.subtract)

            # mem2_1 = cur2_1 already in place.
            for t in range(2, T + 1):
                rp, rt = _row_of(t - 1), _row_of(t)
                vp = m2[:, :, rp:rp + 2]             # mem2_{t-1}
                vt = m2[:, :, rt:rt + 2]             # cur2_t -> mem2_t
                r2 = wp.tile([128, nj, 2], f32d, tag="r2", name=f"r2_{t}")
                u = wp.tile([128, nj, 2], f32d, tag="u", name=f"u_{t}")
                nc.vector.tensor_single_scalar(r2[:], vp, 1.0, Alu.is_gt)
                # u = beta * mem2_{t-1} - reset2_t
                nc.vector.scalar_tensor_tensor(u[:], vp, float(BETA), r2[:],
                                               Alu.mult, Alu.subtract)
                nc.vector.tensor_tensor(vt, vt, u[:], Alu.add)

            # ---- reorder to (t, j, o) and DMA out ------------------------
            src1 = m2[:, :, 4:32].rearrange("p j (t o) -> p t j o", o=2)
            src2 = m2[:, :, 36:58].rearrange("p j (t o) -> p t j o", o=2)
            nc.vector.tensor_copy(osb[:, 0:G1, :, :], src1)
            nc.vector.tensor_copy(osb[:, G1:T, :, :], src2)
            dst = out_d.rearrange("t (p j) o -> p t j o", p=128)
            nc.sync.dma_start(dst, osb[:])
            ps_m2.__exit__(None, None, None)

    nc.compile()
    return nc


def _get_nc(sh, rc, n_iter=1):
    key = (sh, rc, n_iter)
    if key not in _CACHE:
        _CACHE[key] = _build_nc(sh, rc, n_iter)
    return _CACHE[key]


def _host_consts(W1, b1, W2, b2, rc):
    inv = 1.0 / (1.0 - np.float64(BETA))
    iw = np.zeros((NI + 1, 128), f32)
    iw[0:NI, 0:NH] = (-W1.astype(np.float64).T * inv).astype(f32)
    iw[NI, 0:NH] = (-b1.astype(np.float64) * inv).astype(f32)

    S = np.zeros((NH + 1, T * 128), np.float64)
    for t in range(1, T + 1):
        blk = S[:, (t - 1) * 128:t * 128]
        if t < T:
            ct1 = np.float64(BETA) ** -(t + 1)
            idx = np.arange(NH)
            blk[idx, idx] = -ct1
        g = (t - 1) if t <= G1 else (t - 1 - G1)
        blk[0:NH, 100 + 2 * g] = W2[0]
        blk[0:NH, 101 + 2 * g] = W2[1]
        blk[NH, 100 + 2 * g] = b2[0]
        blk[NH, 101 + 2 * g] = b2[1]
    shi_f = _bf16_round(S.astype(f32))
    slo_f = _bf16_round((S - shi_f).astype(f32))
    shi = _bf16_np(shi_f)
    slo = _bf16_np(slo_f)
    ones = _bf16_np(np.ones((1, rc), f32))
    ident = np.eye(128, dtype=f32)
    return iw, shi, slo, ones, ident


def kernel(x, W1, b1, W2, b2):
    global _LAST_RESULT, _LAST_IN_MAPS
    from concourse.bass_utils import run_bass_kernel_spmd

    x = np.ascontiguousarray(x, f32)
    W1 = np.asarray(W1, f32)
    b1 = np.asarray(b1, f32)
    W2 = np.asarray(W2, f32)
    b2 = np.asarray(b2, f32)

    sh, rc = SH, 2048
    nc = _get_nc(sh, rc)
    iw, shi, slo, ones, ident = _host_consts(W1, b1, W2, b2, rc)

    # column c of the device layout holds batch element perm[c]; chosen so the
    # output DMA writes 512B-contiguous DRAM chunks per partition.
    cols = np.arange(sh)
    perm = (cols % 128) * (sh // 128) + cols // 128

    in_maps = []
    for i in range(NCORES):
        xs = x[i * sh:(i + 1) * sh]
        xt = np.ones((NI + 1, sh), f32)
        xt[0:NI] = xs[perm].T
        in_maps.append({"xt": xt, "iw": iw, "shi": shi, "slo": slo,
                        "ones": ones, "ident": ident})

    _LAST_IN_MAPS = in_maps
    res = run_bass_kernel_spmd(nc, in_maps, list(range(NCORES)))
    _LAST_RESULT = res
    return np.concatenate([res.results[i]["out"] for i in range(NCORES)],
                          axis=1)
